# revision 1
# baseline (speedup 1.0000x reference)
"""LIF spike scan kernel for Trainium2 (8 NeuronCores, data-parallel).

Reference computation (per element, scanned over t):
    mem = mem * 0.2 * (1 - spk) + x[t]
    spk = (mem > 0.5)

Carry formulation (v = mem * (mem <= 0.5), the post-reset membrane):
    m   = (v * 0.2) + x[t]   -> DVE scalar_tensor_tensor (split per v half)
    spk = (m > 0.5)          -> ACT u8 = sign(m-0.5) (saturating write
                                clamps -1 to 0: exact {0,1}); packed tiles
                                use DVE tensor_scalar is_gt -> bf16 instead
    v, cols [0,FD)           -> DVE fused scalar_tensor_tensor
    v, cols [FD,F)           -> b = sign(0.5-m) as u8 on ACT (one op, the
                                saturating anti-mask; b=(m<0.5) exactly),
                                then v = b * m on Pool tensor_tensor (the
                                mixed u8xf32 multiply is exact on HW)

The anti-mask removes DVE's per-column comparison, dropping the
recurrence floor to ~2948ns/iter — below the packed-output DMA budget,
which re-opens output compression: q-tiles 1 and 3 pack spikes 8-per-u8
on the PE (weights W[8c+b, 128j + 32j + c] = 2^b land timestep j of a
4-step group at PSUM rows 32j..32j+15; j=0 start=True zero-resets the
region), one ACT fp32->u8 copy per group, and one full-tile [128, 2048]
u8 DMA (the only u8 DMA shape that lowers correctly). Output traffic:
2 x 4.2MB unpacked + 2 x 1.05MB packed = 10.5MB/core (vs 16.8), DMA
floor 215.6us. Tiles 0/2 stay on the single-op u8 sign spike.

Engine budget per iteration pair (one unpacked + one packed iter,
F=2048, FD=512): DVE ~3270, Pool ~3140, ACT ~3070, DMA ~3370 <- bound.

Only m=0.5-exactly deviates from the reference (b=0 there instead of 1,
measure-zero under random normal inputs; comfortably inside the 2e-2
relative tolerance). Everything else is bit-identical fp32.

Sharding: x is [T=16, B=64, C=128, H=32, W=32]; each core takes a
contiguous 1/8 of the flattened B*C*H*W axis viewed as [T, 128, 8192].
Two q-tile chains (one unpacked, one packed) interleave per pass.
"""

import numpy as np

T = 16
SPATIAL = 64 * 128 * 32 * 32  # 8388608
N_CORES = 8
NPC = SPATIAL // N_CORES      # 1048576 elements per core per timestep
P = 128                       # SBUF partitions
Q = NPC // P                  # 8192 free-dim columns per core
F = 2048                      # free-dim tile size
FD = 512                      # v columns on DVE (fused); rest via ACT+Pool
TG = 4                        # timesteps per packed PSUM group
DECAY = 0.2
THRESH = 0.5

_cache = {}

# Set by test harness to request an NTFF trace / HW timing.
TRACE = False


def _pack_weights() -> np.ndarray:
    """[128, 512] fp32, 4 blocks of [128, 128]: W[8c+b, 128j + 32j + c] =
    2^b for c in [0,16), b in [0,8); else 0. Block j lands timestep j's
    packed bytes at PSUM rows 32j..32j+15."""
    w = np.zeros((P, 4 * P), dtype=np.float32)
    for j in range(4):
        for c in range(16):
            for b in range(8):
                w[8 * c + b, P * j + 32 * j + c] = float(1 << b)
    return w


def _build():
    from contextlib import ExitStack

    import concourse.bacc as bacc
    import concourse.tile as tile
    from concourse import mybir

    f32 = mybir.dt.float32
    bf16 = mybir.dt.bfloat16
    u8 = mybir.dt.uint8
    Alu = mybir.AluOpType
    Act = mybir.ActivationFunctionType

    nc = bacc.Bacc("TRN2", target_bir_lowering=False, debug=False)
    x_d = nc.dram_tensor("x", [T, P, Q], f32, kind="ExternalInput").ap()
    w_d = nc.dram_tensor("w", [P, 4 * P], f32, kind="ExternalInput").ap()
    # Unpacked spike planes (q-tiles 0 and 2 only; packed tiles' slices
    # are never written).
    o_d = nc.dram_tensor("spk", [T, P, Q], u8, kind="ExternalOutput").ap()
    # Packed planes: [packed-tile-idx, group, 128, F]; rows 32j+c hold
    # sum_b 2^b * spk[4g+j, 8c+b, col] (rows 16..31 of each block zero).
    p_d = nc.dram_tensor("pck", [2, T // TG, P, F], u8, kind="ExternalOutput").ap()

    # Const APs for activation biases (written pre-tile-region + barrier so
    # bias reads stay untracked).
    _bn = nc.alloc_sbuf_tensor("const-f32-negthresh", [128, 1], f32)
    nc.gpsimd.memset(_bn.ap(), -THRESH)
    nc.const_aps.aps[(f32, -THRESH)] = _bn.ap()
    _bp = nc.alloc_sbuf_tensor("const-f32-posthresh", [128, 1], f32)
    nc.gpsimd.memset(_bp.ap(), THRESH)
    nc.const_aps.aps[(f32, THRESH)] = _bp.ap()
    nc.all_engine_barrier()

    with tile.TileContext(nc) as tc, ExitStack() as ctx:
        wpool = ctx.enter_context(tc.tile_pool(name="wgt", bufs=1))
        xpool = ctx.enter_context(tc.tile_pool(name="xin", bufs=8))
        dpool = ctx.enter_context(tc.tile_pool(name="vd", bufs=5))
        vpool = ctx.enter_context(tc.tile_pool(name="vp", bufs=7))
        bpool = ctx.enter_context(tc.tile_pool(name="ble", bufs=6))
        spool = ctx.enter_context(tc.tile_pool(name="spk", bufs=6))
        opool = ctx.enter_context(tc.tile_pool(name="out", bufs=6))
        ppool = ctx.enter_context(tc.tile_pool(name="acc", bufs=2, space="PSUM"))

        w_f32 = wpool.tile([P, 4 * P], f32)
        nc.sync.dma_start(w_f32[:], w_d)
        wts = []
        for j in range(TG):
            wj = wpool.tile([P, P], bf16, name=f"w{j}")
            nc.scalar.activation(wj[:], w_f32[:, P * j : P * (j + 1)], Act.Copy)
            wts.append(wj)

        pend = []  # deferred packed-group copy+DMA: (tile_idx, g, psum tile)

        def flush(final=False):
            for ti, g, pacc in pend:
                o = opool.tile([P, F], u8, name="po")
                nc.scalar.activation(o[:], pacc[:], Act.Relu)
                nc.scalar.dma_start(p_d[ti, g, :, :], o[:])
            pend.clear()

        # pair p: unpacked tile 2p (chain U) + packed tile 2p+1 (chain K)
        for pair in range(2):
            qU = (2 * pair) * F
            qK = (2 * pair + 1) * F
            v = {q0: (None, None, False) for q0 in (qU, qK)}
            acc = None
            for t in range(T):
                if t % TG == 1 and pend:
                    flush()
                if t % TG == 0:
                    acc = ppool.tile([P, F], f32, name="acc")
                for q0 in (qU, qK):
                    xt = xpool.tile([P, F], f32, name="xt")
                    nc.sync.dma_start(xt[:], x_d[t, :, q0 : q0 + F])
                    m = xt
                    vd, vp, *was_split = v[q0]
                    H = (F - FD) // 2
                    if vd is not None:
                        if was_split and was_split[0]:
                            # tail mode: consume the vp halves independently
                            # so each half of the next step launches as soon
                            # as its own Pool multiply lands.
                            nc.vector.scalar_tensor_tensor(
                                m[:, FD : FD + H], vp[:, 0:H], DECAY,
                                xt[:, FD : FD + H], op0=Alu.mult, op1=Alu.add,
                            )
                            nc.vector.scalar_tensor_tensor(
                                m[:, FD + H : F], vp[:, H:], DECAY,
                                xt[:, FD + H : F], op0=Alu.mult, op1=Alu.add,
                            )
                        else:
                            nc.vector.scalar_tensor_tensor(
                                m[:, FD:F], vp[:], DECAY, xt[:, FD:F],
                                op0=Alu.mult, op1=Alu.add,
                            )
                        nc.vector.scalar_tensor_tensor(
                            m[:, 0:FD], vd[:], DECAY, xt[:, 0:FD],
                            op0=Alu.mult, op1=Alu.add,
                        )
                    if t < T - 1:
                        # anti-mask b = (m < 0.5) in one saturating ACT op;
                        # Pool multiplies the u8 mask straight into m. In the
                        # last few timesteps (the exposed pipeline tail) the
                        # b/multiply pair is emitted per column half so the
                        # serial m->b->mult->m chain pipelines at half-width.
                        split = True
                        b = bpool.tile([P, F - FD], u8, name="b")
                        vpn = vpool.tile([P, F - FD], f32, name="vpn")
                        if split:
                            for h0, h1 in ((0, H), (H, F - FD)):
                                nc.scalar.activation(
                                    b[:, h0:h1], m[:, FD + h0 : FD + h1],
                                    Act.Sign, scale=-1.0, bias=THRESH,
                                )
                                nc.gpsimd.tensor_tensor(
                                    vpn[:, h0:h1], b[:, h0:h1],
                                    m[:, FD + h0 : FD + h1], op=Alu.mult,
                                )
                        else:
                            nc.scalar.activation(
                                b[:], m[:, FD:F], Act.Sign,
                                scale=-1.0, bias=THRESH,
                            )
                            nc.gpsimd.tensor_tensor(
                                vpn[:], b[:], m[:, FD:F], op=Alu.mult
                            )
                        vdn = dpool.tile([P, FD], f32, name="vdn")
                        nc.vector.scalar_tensor_tensor(
                            vdn[:], m[:, 0:FD], THRESH, m[:, 0:FD],
                            op0=Alu.is_le, op1=Alu.mult,
                        )
                        v[q0] = (vdn, vpn, split)
                    if q0 == qU:
                        # unpacked spike: one saturating sign -> u8
                        o = opool.tile([P, F], u8, name="o")
                        nc.scalar.activation(o[:], m[:], Act.Sign, bias=-THRESH)
                        nc.scalar.dma_start(o_d[t, :, q0 : q0 + F], o[:])
                    else:
                        # packed spike: {0,1} bf16 for the PE, then pack 128
                        # rows -> 16 u8-valued PSUM rows at block t%TG.
                        s = spool.tile([P, F], bf16, name="s")
                        nc.vector.tensor_scalar(
                            s[:], m[:], THRESH, None, op0=Alu.is_gt
                        )
                        j = t % TG
                        for c in range(F // 512):
                            ch = slice(512 * c, 512 * (c + 1))
                            nc.tensor.matmul(
                                acc[:, ch], wts[j][:], s[:, ch],
                                start=(j == 0), stop=(j == TG - 1),
                                skip_group_check=True,
                            )
                        if j == TG - 1:
                            pend.append((pair, t // TG, acc))
            flush(final=True)
    nc.compile()
    return nc


def kernel(x: np.ndarray) -> np.ndarray:
    from concourse.bass_utils import run_bass_kernel_spmd

    if "nc" not in _cache:
        _cache["nc"] = _build()
    nc = _cache["nc"]

    x = np.ascontiguousarray(x, dtype=np.float32).reshape(T, N_CORES, NPC)
    w = _pack_weights()
    in_maps = [
        {"x": np.ascontiguousarray(x[:, i]).reshape(T, P, Q), "w": w}
        for i in range(N_CORES)
    ]
    res = run_bass_kernel_spmd(
        nc, in_maps, core_ids=list(range(N_CORES)), trace=TRACE
    )
    _cache["last_results"] = res
    outs = []
    for r in res.results:
        spk = np.asarray(r["spk"]).reshape(T, P, Q).copy()
        pck = np.asarray(r["pck"]).reshape(2, T // TG, P, F)
        for pi in range(2):
            tile0 = (2 * pi + 1) * F
            blk = pck[pi].reshape(T // TG, TG, 32, F)[:, :, 0:16, :]
            bits = np.unpackbits(blk[:, :, :, None, :], axis=3, bitorder="little")
            spk[:, :, tile0 : tile0 + F] = bits.reshape(T, P, F)
        outs.append(spk)
    out = np.stack(outs, axis=1).astype(np.float32).reshape(T, NPC * N_CORES)
    return out.reshape(T, 64, 128, 32, 32)



# revision 20
# speedup vs baseline: 1.0204x; 1.0204x over previous
"""LIF spike scan kernel for Trainium2 (8 NeuronCores, data-parallel).

Reference computation (per element, scanned over t):
    mem = mem * 0.2 * (1 - spk) + x[t]
    spk = (mem > 0.5)

Carry formulation with v = post-reset membrane (v = mem * (mem <= 0.5)):
    m = v * 0.2 + x[t]          -> DVE scalar_tensor_tensor (1x, split in two
                                   column ranges for pipelining)
    a = (m <= 0.5) in {0,1}     -> anti-spike mask. Columns [0,FD): DVE
                                   tensor_scalar is_le -> bf16 (2x perf mode).
                                   Columns [FD,F): ACT Sign(0.5-m) -> u8
                                   (saturating, exact {0,1}) + ACT Copy u8->bf16.
    v = min(m, a)               -> Pool tensor_tensor min (eff 0.58, cheaper
                                   than mult): a=1 -> m (m<=0.5<1), a=0 -> 0
                                   (m>0.5>0). Exact, including m==0.5 on the
                                   DVE columns.

Output is fully bit-packed on the PE: weight block j (j = t mod 8) has
W[8c+b, 16j+c] = 2^b, so after 8 accumulating matmuls PSUM row 16j+c holds
sum_b 2^b * a[8c+b] for timestep j -- all 128 rows useful (zero padding
waste). One ACT copy PSUM->u8 with scale=-1, bias=255 per group flips the
anti-mask packing into true packed spikes (255 - sum 2^b*(1-s) = sum 2^b*s),
then one [128, 2048] u8 DMA per 8 timesteps. Output traffic: 2.1 MB/core
(vs 67.1 MB input).

Engine budget per tile-iteration (F=2048 cols, 64 iterations/core):
DVE ~3030ns, Pool ~3035, DMA ~3005 (in 2913 + out 91), ACT ~1720,
PE ~1710 -> DMA/DVE/Pool co-bound near the 360 GB/s HBM roofline (~194us).

Only m==0.5-exactly deviates from the reference on the ACT columns
(measure-zero under random normal inputs). Everything else is bit-exact fp32.

Sharding: x is [T=16, B=64, C=128, H=32, W=32]; each core takes a contiguous
1/8 of the flattened B*C*H*W axis viewed as [T, 128, 8192]. Two q-tile
chains run concurrently per pass (PSUM holds two [128,2048] f32 group
accumulators); two passes cover the four q-tiles.
"""

import numpy as np

T = 16
SPATIAL = 64 * 128 * 32 * 32  # 8388608
N_CORES = 8
NPC = SPATIAL // N_CORES      # 1048576 elements per core per timestep
P = 128                       # SBUF partitions
Q = NPC // P                  # 8192 free-dim columns per core
F = 2048                      # free-dim tile size
A = 1504                      # ACT-mask + Pool-mult columns; rest DVE isle+vdn
A1 = 752                      # Pool mult / sign / stt split inside [0, A)
TG = 8                        # timesteps per packed PSUM group
NG = T // TG                  # groups per tile
DECAY = 0.2
THRESH = 0.5

_cache = {}

# Set by test harness to request an NTFF trace / HW timing.
TRACE = False


def _pack_weights() -> np.ndarray:
    """[128, 1024] fp32, 8 blocks of [128, 128]: block j has
    W[8c+b, 128j + 16j + c] = 2^b for c in [0,16), b in [0,8); else 0.
    Block j lands timestep j's packed bytes at PSUM rows 16j..16j+15."""
    w = np.zeros((P, TG * P), dtype=np.float32)
    for j in range(TG):
        for c in range(16):
            for b in range(8):
                w[8 * c + b, P * j + 16 * j + c] = float(1 << b)
    return w


def _build():
    from contextlib import ExitStack

    import concourse.bacc as bacc
    import concourse.tile as tile
    from concourse import mybir

    f32 = mybir.dt.float32
    bf16 = mybir.dt.bfloat16
    u8 = mybir.dt.uint8
    Alu = mybir.AluOpType
    Act = mybir.ActivationFunctionType

    nc = bacc.Bacc("TRN2", target_bir_lowering=False, debug=False)
    x_d = nc.dram_tensor("x", [T, P, Q], f32, kind="ExternalInput").ap()
    w_d = nc.dram_tensor("w", [P, TG * P], f32, kind="ExternalInput").ap()
    # Packed planes: [q-tile, group, 128, F]; row 16j+c of group g holds
    # sum_b 2^b * spk[8g+j, 8c+b, col].
    p_d = nc.dram_tensor("pck", [4, NG, P, F], u8, kind="ExternalOutput").ap()

    # Const APs for activation biases (written pre-tile-region + barrier so
    # bias reads stay untracked).
    for name, val in (("thr", THRESH), ("zero", 0.0), ("flip", 255.0)):
        ap = nc.alloc_sbuf_tensor(f"const-f32-{name}", [128, 1], f32)
        nc.gpsimd.memset(ap.ap(), val)
        nc.const_aps.aps[(f32, val)] = ap.ap()
    nc.all_engine_barrier()

    with tile.TileContext(nc) as tc, ExitStack() as ctx:
        wpool = ctx.enter_context(tc.tile_pool(name="wgt", bufs=1))
        xpool = ctx.enter_context(tc.tile_pool(name="xin", bufs=12))
        vpool = ctx.enter_context(tc.tile_pool(name="vst", bufs=5))
        apool = ctx.enter_context(tc.tile_pool(name="ams", bufs=7))
        bpool = ctx.enter_context(tc.tile_pool(name="bms", bufs=7))
        opool = ctx.enter_context(tc.tile_pool(name="out", bufs=4))
        ppool = ctx.enter_context(tc.tile_pool(name="acc", bufs=2, space="PSUM"))

        w_f32 = wpool.tile([P, TG * P], f32)
        wb = wpool.tile([P, TG * P], bf16, name="wb")

        pend = []  # deferred flip+store: (ti, group, psum tile)

        def flush(n=None):
            todo = pend[:n] if n else pend[:]
            del pend[: len(todo)]
            for ti, g, pacc in todo:
                o = opool.tile([P, F], u8, name="po")
                nc.scalar.activation(o[:], pacc[:], Act.Copy, scale=-1.0, bias=255.0)
                nc.scalar.dma_start(p_d[ti, g, :, :], o[:])

        first = True
        state = {}

        def load(ti, t):
            q0 = ti * F
            xt = xpool.tile([P, F], f32, name="xt")
            nc.sync.dma_start(xt[:], x_d[t, :, q0 : q0 + F])
            return xt

        def front(ti, t, vprev, xt=None):
            """x load + m-update + masks + reset for (ti, t); mats deferred."""
            nonlocal first
            if xt is None:
                xt = load(ti, t)
            if first:
                # weight load queued behind the first x tile so x[0] isn't
                # delayed; the bf16 copy lands well before the first matmul
                nc.sync.dma_start(w_f32[:], w_d)
                nc.scalar.activation(wb[:], w_f32[:], Act.Copy)
                first = False
            m = xt
            a = apool.tile([P, F], bf16, name="a")
            b = bpool.tile([P, A], u8, name="b")
            vn = vpool.tile([P, F], f32, name="vn") if t < T - 1 else None
            # m-update in three ops aligned with the apply ranges so each
            # only waits its own v producer
            if vprev is not None:
                for r0, r1 in ((0, A1), (A1, A), (A, F)):
                    nc.vector.scalar_tensor_tensor(
                        m[:, r0:r1], vprev[:, r0:r1], DECAY,
                        m[:, r0:r1], op0=Alu.mult, op1=Alu.add,
                    )
            # anti-spike mask: [0,A) ACT Sign(0.5-m) -> u8 {0,1} (saturating,
            # split at A1 so each Pool mult waits only its own half);
            # [A,F) DVE is_le -> bf16 (2x mode)
            nc.scalar.activation(
                b[:, 0:A1], m[:, 0:A1], Act.Sign, scale=-1.0, bias=THRESH
            )
            nc.scalar.activation(
                b[:, A1:A], m[:, A1:A], Act.Sign, scale=-1.0, bias=THRESH
            )
            if vn is not None:
                # hard reset v = mask * m. The u8 b feeds Pool directly
                # (exact {0,1} multiply), keeping the bf16 copy off the
                # recurrence critical path.
                nc.gpsimd.tensor_tensor(
                    vn[:, 0:A1], b[:, 0:A1], m[:, 0:A1], op=Alu.mult
                )
                nc.gpsimd.tensor_tensor(
                    vn[:, A1:A], b[:, A1:A], m[:, A1:A], op=Alu.mult
                )
                # [A,F): fused compare+multiply on DVE
                nc.vector.scalar_tensor_tensor(
                    vn[:, A:F], m[:, A:F], THRESH, m[:, A:F],
                    op0=Alu.is_le, op1=Alu.mult,
                )
            nc.vector.tensor_scalar(
                a[:, A:F], m[:, A:F], THRESH, None, op0=Alu.is_le
            )
            nc.scalar.activation(a[:, 0:A], b[:], Act.Copy)
            return a, vn

        def mats(ti, t, a, acc):
            """flip boundary + pack matmuls for (ti, t); returns acc."""
            j = t % TG
            if j == 0:
                # flip+store the tile's previous group after this step's
                # masks (no ACT head-of-line burst at the boundary), then
                # reallocate its PSUM accumulator (matmuls below wait on the
                # flip via the pool slot)
                if acc is not None:
                    pend.append((ti, t // TG - 1, acc))
                    flush()
                acc = ppool.tile([P, F], f32, name="acc")
            for c in range(F // 512):
                ch = slice(512 * c, 512 * (c + 1))
                nc.tensor.matmul(
                    acc[:, ch], wb[:, P * j : P * (j + 1)], a[:, ch],
                    start=(j == 0), stop=(j == TG - 1),
                    skip_group_check=True,
                )
            return acc

        for pair in range(2):
            tiles = (2 * pair, 2 * pair + 1)
            v = {ti: None for ti in tiles}
            acc = {ti: None for ti in tiles}
            for ti, t in [(ti, t) for t in range(T) for ti in tiles]:
                if (pair, ti, t) in state:
                    a, vn = state.pop((pair, ti, t))
                else:
                    a, vn = front(ti, t, v[ti])
                v[ti] = vn
                acc[ti] = mats(ti, t, a, acc[ti])
                if pair == 0 and t == T - 3 and ti == tiles[1]:
                    # software-pipeline the pass boundary: the next pass's
                    # t=0 has no recurrence inputs, so its load + masks +
                    # reset run under this pass's tail; only its matmuls
                    # wait (on the PSUM flips) at the real boundary
                    for nti in (tiles[0] + 2, tiles[1] + 2):
                        state[(1, nti, 0)] = front(nti, 0, None)
            for ti in tiles:
                pend.append((ti, (T - 1) // TG, acc[ti]))
            flush()
    nc.compile()
    return nc


def kernel(x: np.ndarray) -> np.ndarray:
    from concourse.bass_utils import run_bass_kernel_spmd

    if "nc" not in _cache:
        _cache["nc"] = _build()
    nc = _cache["nc"]

    x = np.ascontiguousarray(x, dtype=np.float32).reshape(T, N_CORES, NPC)
    w = _pack_weights()
    in_maps = [
        {"x": np.ascontiguousarray(x[:, i]).reshape(T, P, Q), "w": w}
        for i in range(N_CORES)
    ]
    res = run_bass_kernel_spmd(
        nc, in_maps, core_ids=list(range(N_CORES)), trace=TRACE
    )
    _cache["last_results"] = res
    outs = []
    for r in res.results:
        pck = np.asarray(r["pck"]).reshape(4, NG, P, F)
        # row 16j+c of group g = packed spikes for t=8g+j, partitions 8c+b
        blk = pck.reshape(4, NG, TG, 16, F)
        bits = np.unpackbits(blk[:, :, :, :, None, :], axis=4, bitorder="little")
        # bits: [tile, g, j, c, b, F] -> [g, j, c, b, tile, F] -> [T, P, Q]
        spk = bits.transpose(1, 2, 3, 4, 0, 5).reshape(T, P, Q)
        outs.append(spk)
    out = np.stack(outs, axis=1).astype(np.float32).reshape(T, NPC * N_CORES)
    return out.reshape(T, 64, 128, 32, 32)


# revision 23
# speedup vs baseline: 1.0255x; 1.0050x over previous
"""LIF spike scan kernel for Trainium2 (8 NeuronCores, data-parallel).

Reference computation (per element, scanned over t):
    mem = mem * 0.2 * (1 - spk) + x[t]
    spk = (mem > 0.5)

Carry formulation with v = post-reset membrane (v = mem * (mem <= 0.5)):
    m = v * 0.2 + x[t]    -> DVE scalar_tensor_tensor, three ops aligned with
                             the v-producer ranges below
    mask + reset, columns [0, A) ("B path"):
        b = (m < 0.5)     -> ACT Sign(0.5-m) -> saturating u8, exact {0,1};
                             split at A1 so each Pool mult waits only its half
        v = b * m         -> Pool tensor_tensor mult (u8 x f32, exact),
                             two ops split at A1
        a = b             -> ACT Copy u8 -> bf16 (pack operand; off the
                             recurrence critical path)
    mask + reset, columns [A, F) ("D path", all-DVE):
        a = (m <= 0.5)    -> DVE tensor_scalar is_le -> bf16 (2x perf mode)
        v = (m <= 0.5)*m  -> DVE fused scalar_tensor_tensor is_le+mult

Output is fully bit-packed on the PE: weight block j (j = t mod 8) has
W[8c+k, 16j+c] = 2^k, so after 8 accumulating matmuls PSUM row 16j+c holds
sum_k 2^k * a[8c+k] for timestep j -- all 128 rows useful (zero padding
waste). One ACT copy PSUM->u8 with scale=-1, bias=255 per group flips the
anti-mask packing into true packed spikes (255 - sum 2^k*(1-s) = sum 2^k*s),
then one [128, 2048] u8 DMA per 8 timesteps. Output traffic: 2.1 MB/core
(vs 67.1 MB input; the baseline wrote 10.5 MB).

Engine busy per tile-iteration (F=2048 cols, 64 iterations/core):
DVE ~3290ns, ACT ~3290, Pool ~3170, DMA ~2980 (in 2913 + out 46), PE ~900
-> the three compute engines are co-bound just above the 360 GB/s HBM
input stream; Pool min/max and gpsimd stt are not available on trn2 HW
(ISA check), which is why the reset costs a mult + two mask producers.
Group flips are emitted after both tiles' mask ops (ACT head-of-line) and
the second pass's t=0 front is software-pipelined under the first pass's
tail (its matmuls wait on the PSUM flips at the real boundary).

Only m == 0.5 exactly deviates from the reference on the B-path columns
(spike emitted instead of held; measure-zero under random normal inputs --
4 of 134M elements on the seed-0 input). Everything else is bit-exact fp32.

Sharding: x is [T=16, B=64, C=128, H=32, W=32]; each core takes a contiguous
1/8 of the flattened B*C*H*W axis viewed as [T, 128, 8192]. Two q-tile
chains run concurrently per pass (PSUM holds two [128, 2048] f32 group
accumulators); two passes cover the four q-tiles.
"""

import numpy as np

T = 16
SPATIAL = 64 * 128 * 32 * 32  # 8388608
N_CORES = 8
NPC = SPATIAL // N_CORES      # 1048576 elements per core per timestep
P = 128                       # SBUF partitions
Q = NPC // P                  # 8192 free-dim columns per core
F = 2048                      # free-dim tile size
A = 1472                      # ACT-mask + Pool-mult columns; rest DVE isle+vdn
A1 = 704                      # Pool mult / sign / stt split inside [0, A)
TG = 8                        # timesteps per packed PSUM group
NG = T // TG                  # groups per tile
DECAY = 0.2
THRESH = 0.5

_cache = {}

# Set by test harness to request an NTFF trace / HW timing.
TRACE = False


def _pack_weights() -> np.ndarray:
    """[128, 1024] fp32, 8 blocks of [128, 128]: block j has
    W[8c+b, 128j + 16j + c] = 2^b for c in [0,16), b in [0,8); else 0.
    Block j lands timestep j's packed bytes at PSUM rows 16j..16j+15."""
    w = np.zeros((P, TG * P), dtype=np.float32)
    for j in range(TG):
        for c in range(16):
            for b in range(8):
                w[8 * c + b, P * j + 16 * j + c] = float(1 << b)
    return w


def _build():
    from contextlib import ExitStack

    import concourse.bacc as bacc
    import concourse.tile as tile
    from concourse import mybir

    f32 = mybir.dt.float32
    bf16 = mybir.dt.bfloat16
    u8 = mybir.dt.uint8
    Alu = mybir.AluOpType
    Act = mybir.ActivationFunctionType

    nc = bacc.Bacc("TRN2", target_bir_lowering=False, debug=False)
    x_d = nc.dram_tensor("x", [T, P, Q], f32, kind="ExternalInput").ap()
    w_d = nc.dram_tensor("w", [P, TG * P], f32, kind="ExternalInput").ap()
    # Packed planes: [q-tile, group, 128, F]; row 16j+c of group g holds
    # sum_b 2^b * spk[8g+j, 8c+b, col].
    p_d = nc.dram_tensor("pck", [4, NG, P, F], u8, kind="ExternalOutput").ap()

    # Const APs for activation biases (written pre-tile-region + barrier so
    # bias reads stay untracked).
    for name, val in (("thr", THRESH), ("zero", 0.0), ("flip", 255.0)):
        ap = nc.alloc_sbuf_tensor(f"const-f32-{name}", [128, 1], f32)
        nc.gpsimd.memset(ap.ap(), val)
        nc.const_aps.aps[(f32, val)] = ap.ap()
    nc.all_engine_barrier()

    with tile.TileContext(nc) as tc, ExitStack() as ctx:
        wpool = ctx.enter_context(tc.tile_pool(name="wgt", bufs=1))
        xpool = ctx.enter_context(tc.tile_pool(name="xin", bufs=12))
        vpool = ctx.enter_context(tc.tile_pool(name="vst", bufs=5))
        apool = ctx.enter_context(tc.tile_pool(name="ams", bufs=8))
        bpool = ctx.enter_context(tc.tile_pool(name="bms", bufs=7))
        opool = ctx.enter_context(tc.tile_pool(name="out", bufs=4))
        ppool = ctx.enter_context(tc.tile_pool(name="acc", bufs=2, space="PSUM"))

        w_f32 = wpool.tile([P, TG * P], f32)
        wb = wpool.tile([P, TG * P], bf16, name="wb")

        pend = []  # deferred flip+store: (ti, group, psum tile)

        def flush(n=None):
            todo = pend[:n] if n else pend[:]
            del pend[: len(todo)]
            for ti, g, pacc in todo:
                o = opool.tile([P, F], u8, name="po")
                nc.scalar.activation(o[:], pacc[:], Act.Copy, scale=-1.0, bias=255.0)
                nc.scalar.dma_start(p_d[ti, g, :, :], o[:])

        first = True
        state = {}

        def load(ti, t):
            q0 = ti * F
            xt = xpool.tile([P, F], f32, name="xt")
            nc.sync.dma_start(xt[:], x_d[t, :, q0 : q0 + F])
            return xt

        def front(ti, t, vprev, xt=None):
            """x load + m-update + masks + reset for (ti, t); mats deferred."""
            nonlocal first
            if xt is None:
                xt = load(ti, t)
            if first:
                # weight load queued behind the first x tile so x[0] isn't
                # delayed; the bf16 copy lands well before the first matmul
                nc.sync.dma_start(w_f32[:], w_d)
                nc.scalar.activation(wb[:], w_f32[:], Act.Copy)
                first = False
            m = xt
            a = apool.tile([P, F], bf16, name="a")
            b = bpool.tile([P, A], u8, name="b")
            vn = vpool.tile([P, F], f32, name="vn") if t < T - 1 else None
            # m-update in three ops aligned with the apply ranges so each
            # only waits its own v producer
            if vprev is not None:
                for r0, r1 in ((0, A1), (A1, A), (A, F)):
                    nc.vector.scalar_tensor_tensor(
                        m[:, r0:r1], vprev[:, r0:r1], DECAY,
                        m[:, r0:r1], op0=Alu.mult, op1=Alu.add,
                    )
            # anti-spike mask: [0,A) ACT Sign(0.5-m) -> u8 {0,1} (saturating,
            # split at A1 so each Pool mult waits only its own half);
            # [A,F) DVE is_le -> bf16 (2x mode)
            nc.scalar.activation(
                b[:, 0:A1], m[:, 0:A1], Act.Sign, scale=-1.0, bias=THRESH
            )
            nc.scalar.activation(
                b[:, A1:A], m[:, A1:A], Act.Sign, scale=-1.0, bias=THRESH
            )
            if vn is not None:
                # hard reset v = mask * m. The u8 b feeds Pool directly
                # (exact {0,1} multiply), keeping the bf16 copy off the
                # recurrence critical path.
                nc.gpsimd.tensor_tensor(
                    vn[:, 0:A1], b[:, 0:A1], m[:, 0:A1], op=Alu.mult
                )
                nc.gpsimd.tensor_tensor(
                    vn[:, A1:A], b[:, A1:A], m[:, A1:A], op=Alu.mult
                )
                # [A,F): fused compare+multiply on DVE
                nc.vector.scalar_tensor_tensor(
                    vn[:, A:F], m[:, A:F], THRESH, m[:, A:F],
                    op0=Alu.is_le, op1=Alu.mult,
                )
            nc.vector.tensor_scalar(
                a[:, A:F], m[:, A:F], THRESH, None, op0=Alu.is_le
            )
            nc.scalar.activation(a[:, 0:A], b[:], Act.Copy)
            return a, vn

        def mats(ti, t, a, acc):
            """pack matmuls for (ti, t) into acc."""
            j = t % TG
            for c in range(F // 512):
                ch = slice(512 * c, 512 * (c + 1))
                nc.tensor.matmul(
                    acc[:, ch], wb[:, P * j : P * (j + 1)], a[:, ch],
                    start=(j == 0), stop=(j == TG - 1),
                    skip_group_check=True,
                )
            return acc

        for pair in range(2):
            tiles = (2 * pair, 2 * pair + 1)
            v = {ti: None for ti in tiles}
            acc = {ti: None for ti in tiles}
            for t in range(T):
                j = t % TG
                am = {}
                for ti in tiles:
                    if (pair, ti, t) in state:
                        a, vn = state.pop((pair, ti, t))
                    else:
                        a, vn = front(ti, t, v[ti])
                    v[ti] = vn
                    am[ti] = a
                if j == 0:
                    # flip+store the previous groups after BOTH tiles' mask
                    # ops (no ACT head-of-line burst at the boundary), then
                    # reallocate the PSUM accumulators (matmuls below wait
                    # on the flips via the pool slots)
                    for ti in tiles:
                        if acc[ti] is not None:
                            pend.append((ti, t // TG - 1, acc[ti]))
                    flush()
                    for ti in tiles:
                        acc[ti] = ppool.tile([P, F], f32, name="acc")
                for ti in tiles:
                    mats(ti, t, am[ti], acc[ti])
                if pair == 0 and t == T - 3:
                    # software-pipeline the pass boundary: the next pass's
                    # t=0 has no recurrence inputs, so its load + masks +
                    # reset run under this pass's tail; only its matmuls
                    # wait (on the PSUM flips) at the real boundary
                    for nti in (tiles[0] + 2, tiles[1] + 2):
                        state[(1, nti, 0)] = front(nti, 0, None)
            for ti in tiles:
                pend.append((ti, (T - 1) // TG, acc[ti]))
            flush()
    nc.compile()
    return nc


def kernel(x: np.ndarray) -> np.ndarray:
    from concourse.bass_utils import run_bass_kernel_spmd

    if "nc" not in _cache:
        _cache["nc"] = _build()
    nc = _cache["nc"]

    x = np.ascontiguousarray(x, dtype=np.float32).reshape(T, N_CORES, NPC)
    w = _pack_weights()
    in_maps = [
        {"x": np.ascontiguousarray(x[:, i]).reshape(T, P, Q), "w": w}
        for i in range(N_CORES)
    ]
    res = run_bass_kernel_spmd(
        nc, in_maps, core_ids=list(range(N_CORES)), trace=TRACE
    )
    _cache["last_results"] = res
    outs = []
    for r in res.results:
        pck = np.asarray(r["pck"]).reshape(4, NG, P, F)
        # row 16j+c of group g = packed spikes for t=8g+j, partitions 8c+b
        blk = pck.reshape(4, NG, TG, 16, F)
        bits = np.unpackbits(blk[:, :, :, :, None, :], axis=4, bitorder="little")
        # bits: [tile, g, j, c, b, F] -> [g, j, c, b, tile, F] -> [T, P, Q]
        spk = bits.transpose(1, 2, 3, 4, 0, 5).reshape(T, P, Q)
        outs.append(spk)
    out = np.stack(outs, axis=1).astype(np.float32).reshape(T, NPC * N_CORES)
    return out.reshape(T, 64, 128, 32, 32)
